# revision 4
# baseline (speedup 1.0000x reference)
"""CropSplit (SipMask crop-split gather) Trainium2 kernel — wsum variant.

Reference semantics (c=2): for each ROI n and pixel (h, w),
  out[h,w,n] = inside_box ? data[cell(h,w,n), h, w, n] : 0
where cell = yy*2+xx picks one of the 4 mask-basis planes.

Strategy:
  - Shard H (200 rows) across 8 NeuronCores, 25 rows each; per-core
    tensors are contiguous, so device DMAs are large contiguous strips.
  - Host converts the mask-basis planes to bf16 once (pure selection
    tolerates this; harness gate is rel err < 2e-2, bf16 gives ~2e-3)
    and expands rois into one per-element cell code mcell in {0..3},
    4 = outside-box (bf16 so every DVE op runs in a fast perf mode).
  - Device computes out = sum_k (mcell == k) * d_k:
    4x tensor_scalar is_equal (DVE 4x mode) + 4x TT mult + 3x TT add
    (DVE 2x mode). Exactly one term is nonzero per element, so the
    selection is exact in bf16; outside-box elements give 0 without a
    separate zeroing op.
  - fd=6400 tiles halve the DMA-descriptor count vs fd=3200.
"""

import sys

for _p in ("/opt/trn_rl_repo", "/opt/pypackages"):
    if _p not in sys.path:
        sys.path.append(_p)

import numpy as np
import ml_dtypes

BF16 = ml_dtypes.bfloat16

N_CORES = 8
CC, H, W, N = 4, 200, 200, 400
HS = H // N_CORES          # 25 rows per core
ELEMS = HS * W * N         # 2_000_000 elements per plane per core
FD = 8000

_BLOCKS = [(0, 128, 8000), (1024000, 122, 8000)]

_CACHE = {}


def _build_program(repeats: int = 1, bufs: int = 2, fd: int = FD):
    import concourse.bacc as bacc
    import concourse.mybir as mybir
    import concourse.tile as tile

    nc = bacc.Bacc(
        "TRN2",
        target_bir_lowering=False,
        debug=False,
        enable_asserts=False,
        num_devices=N_CORES,
    )
    bf16 = mybir.dt.bfloat16
    A = mybir.AluOpType
    d_in = nc.dram_tensor("data", [CC, ELEMS], bf16, kind="ExternalInput").ap()
    m_in = nc.dram_tensor("mcell", [ELEMS], bf16, kind="ExternalInput").ap()
    o_out = nc.dram_tensor("out", [ELEMS], bf16, kind="ExternalOutput").ap()

    def assign(name):
        return {
            "d0": nc.sync, "d1": nc.scalar, "d2": nc.sync, "d3": nc.scalar,
            "mcell": nc.gpsimd, "out": nc.gpsimd,
        }[name]

    with tile.TileContext(nc) as tc:
        with tc.tile_pool(name="pool", bufs=bufs) as pool:
            for off, p, bfd in _BLOCKS * repeats:
                sz = p * bfd
                ts = []
                for k in range(CC):
                    t = pool.tile([128, fd], bf16, tag=f"d{k}")
                    assign(f"d{k}").dma_start(
                        out=t[:p, :bfd],
                        in_=d_in[k, off : off + sz].rearrange("(p f) -> p f", f=bfd),
                    )
                    ts.append(t)
                tmc = pool.tile([128, fd], bf16, tag="mc")
                assign("mcell").dma_start(
                    out=tmc[:p, :bfd],
                    in_=m_in[off : off + sz].rearrange("(p f) -> p f", f=bfd),
                )
                for k in range(CC):
                    wk = pool.tile([128, fd], bf16, tag="w")
                    nc.vector.tensor_scalar(
                        wk[:p, :bfd], tmc[:p, :bfd], float(k), None, op0=A.is_equal
                    )
                    nc.vector.tensor_tensor(
                        ts[k][:p, :bfd], ts[k][:p, :bfd], wk[:p, :bfd], op=A.mult
                    )
                nc.vector.tensor_tensor(
                    ts[0][:p, :bfd], ts[0][:p, :bfd], ts[1][:p, :bfd], op=A.add
                )
                nc.vector.tensor_tensor(
                    ts[2][:p, :bfd], ts[2][:p, :bfd], ts[3][:p, :bfd], op=A.add
                )
                nc.vector.tensor_tensor(
                    ts[0][:p, :bfd], ts[0][:p, :bfd], ts[2][:p, :bfd], op=A.add
                )
                assign("out").dma_start(
                    out=o_out[off : off + sz].rearrange("(p f) -> p f", f=bfd),
                    in_=ts[0][:p, :bfd],
                )
    nc.compile()
    return nc


def _host_masks(rois: np.ndarray, c: int):
    """Bit-exact float32 replication of the reference cell/inside math."""
    assert c == 2
    x1 = rois[:, 0].astype(np.float32)
    y1 = rois[:, 1].astype(np.float32)
    x2 = rois[:, 2].astype(np.float32)
    y2 = rois[:, 3].astype(np.float32)
    xs = np.arange(W, dtype=np.float32)[:, None]  # [W, 1]
    ys = np.arange(H, dtype=np.float32)[:, None]  # [H, 1]
    bw = np.maximum(x2 - x1, np.float32(1e-6))[None, :]  # [1, N]
    bh = np.maximum(y2 - y1, np.float32(1e-6))[None, :]
    cf = np.float32(c)
    xx = np.clip(np.floor((xs - x1[None, :]) / bw * cf), 0.0, cf - 1.0)  # [W,N] f32
    yy = np.clip(np.floor((ys - y1[None, :]) / bh * cf), 0.0, cf - 1.0)  # [H,N]
    in_x = (xs >= x1[None, :]) & (xs <= x2[None, :])  # [W, N]
    in_y = (ys >= y1[None, :]) & (ys <= y2[None, :])  # [H, N]
    return xx.astype(np.uint8), yy.astype(np.uint8), in_x, in_y


def _make_in_maps(data: np.ndarray, rois: np.ndarray):
    """Per-core input dicts: bf16 data slice + bf16 cell code (4=outside)."""
    xx, yy, in_x, in_y = _host_masks(np.asarray(rois, dtype=np.float32), 2)
    data_bf = np.ascontiguousarray(data, dtype=np.float32).astype(BF16)
    in_maps = []
    for core in range(N_CORES):
        h0, h1 = core * HS, (core + 1) * HS
        cell = (
            np.broadcast_to(xx[None, :, :], (HS, W, N)).astype(np.uint8)
            | (np.broadcast_to((yy[h0:h1])[:, None, :], (HS, W, N)) << 1)
        )
        outside = ~(in_x[None, :, :] & in_y[h0:h1, None, :])
        mcell = np.where(outside, np.uint8(4), cell).astype(BF16)
        in_maps.append(
            {
                "data": np.ascontiguousarray(data_bf[:, h0:h1]).reshape(CC, ELEMS),
                "mcell": mcell.reshape(ELEMS),
            }
        )
    return in_maps


def kernel(data: np.ndarray, rois: np.ndarray, c) -> np.ndarray:
    from concourse.bass_utils import run_bass_kernel_spmd

    c = int(c)
    assert c == 2 and data.shape == (CC, H, W, N)
    in_maps = _make_in_maps(data, rois)

    if "nc" not in _CACHE:
        _CACHE["nc"] = _build_program()
    nc = _CACHE["nc"]

    res = run_bass_kernel_spmd(nc, in_maps, list(range(N_CORES)))
    out = np.empty((H, W, N), dtype=np.float32)
    for core in range(N_CORES):
        h0 = core * HS
        out[h0 : h0 + HS] = res.results[core]["out"].reshape(HS, W, N).astype(np.float32)
    return out
